# revision 29
# baseline (speedup 1.0000x reference)
"""KSG mutual-information estimator (ClusterMI) on 8 Trainium2 NeuronCores.

Math (see reference):
  d2(i,j) = |x_i - x_j|^2 ; same-class 4th-smallest (k=3, self included) gives
  per-row radius; m_i = #{j : d2(i,j) <= radius_i} - 1 ;
  out = max((psi(N) - sum_c (N_c/N) psi(N_c) + psi(3) - mean_i psi(m_i)) / ln 2, 0)

Device strategy (rows sharded 1024/core, X replicated):
  Work in the s' = 2 x_i . x_j - |x_j|^2 domain: per-row ordering of s' is the
  reverse of d2's (row-constant shift), so the masked 4th-largest s' (hardware
  max8) is directly the count threshold: m_i = #{j : s'_ij > t_i - eps} - 1.
  Phase 1 per 128-row block: matmul over a ~1792-wide same-class column window
  (host-packed; cross-class pairs pushed to -BIG via a rank-2 one-hot term in
  the K=4 aux matmul), ScalarE copy PSUM->SBUF, DVE max8 -> threshold.
  Phase 2 per block: K=128 main + K=2 (-sq_hi,-sq_lo) matmuls over all 8192
  cols; counts fused into the single PSUM read: ScalarE Sign+accumulate on even
  2048-col chunks, DVE tensor_scalar(is_gt)+accumulate on odd chunks.
  Then digamma(m) by asymptotic series on-device, partition-sum via a K=128
  N=1 fp32 matmul, one scalar out per core; host combines.

bf16 matmul noise analysis: d2 noise ~0.05 abs is symmetric; count flips are
zero-mean with sigma(avg psi(m)) ~1e-4, far inside the -0.00946 pre-clamp
margin of the reference value (output is exactly 0.0 unless mi noise > 9e-3).
"""

import numpy as np
import ml_dtypes

N = 8192
D = 128
NCORES = 8
ROWS = N // NCORES          # 1024 rows per core
BLOCKS = ROWS // 128        # 8 row-blocks per core
NBLK = N // 128             # 64 global row-blocks
KNN = 3
NCLASSES = 10
P1W = 1792                  # phase-1 window width (max class pair 874+874=1748)
CHUNK = 2048                # phase-2 consumer chunk (4 PSUM banks)
MMN = 512                   # matmul free-dim per instruction
BIG = 30000.0               # cross-class penalty in s' domain
PADV = -3.0e7               # phase-1 pad-column value via -sq row
EPS = 3e-4                  # threshold shift so the anchor itself is counted

bf16 = ml_dtypes.bfloat16

_cache = {}


def _build_nc():
    from contextlib import ExitStack

    import concourse.bass as bass
    import concourse.mybir as mybir
    import concourse.tile as tile

    dt = mybir.dt
    AF = mybir.ActivationFunctionType
    OP = mybir.AluOpType
    AX = mybir.AxisListType

    nc = bass.Bass("TRN2", target_bir_lowering=False, debug=False)

    xt_d = nc.dram_tensor("xt", [D, N], dt.bfloat16, kind="ExternalInput")
    auxr_d = nc.dram_tensor("auxr", [2, N], dt.bfloat16, kind="ExternalInput")
    lhs_d = nc.dram_tensor("lhs", [D, ROWS], dt.bfloat16, kind="ExternalInput")
    p1r_d = nc.dram_tensor("p1r", [BLOCKS, D, P1W], dt.bfloat16, kind="ExternalInput")
    p1ar_d = nc.dram_tensor("p1ar", [BLOCKS, 4, P1W], dt.bfloat16, kind="ExternalInput")
    p1al_d = nc.dram_tensor("p1al", [BLOCKS, 128, D], dt.bfloat16, kind="ExternalInput")
    ones4_d = nc.dram_tensor("ones4", [128, D], dt.bfloat16, kind="ExternalInput")
    onesf_d = nc.dram_tensor("onesf", [D, 1], dt.float32, kind="ExternalInput")
    dsum_d = nc.dram_tensor("dsum", [1, 1], dt.float32, kind="ExternalOutput")
    mout_d = nc.dram_tensor("mout", [128, BLOCKS], dt.float32, kind="ExternalOutput")
    aout_d = nc.dram_tensor("aout", [128, BLOCKS], dt.float32, kind="ExternalOutput")

    POS = (0, 32, 64, 96)

    with tile.TileContext(nc) as tc, ExitStack() as ctx:
        consts = ctx.enter_context(tc.tile_pool(name="consts", bufs=1))
        p1pool = ctx.enter_context(tc.tile_pool(name="p1", bufs=4))
        psum = ctx.enter_context(tc.tile_pool(name="psum", bufs=2, space="PSUM"))
        work = ctx.enter_context(tc.tile_pool(name="work", bufs=2))
        scrp = ctx.enter_context(tc.tile_pool(name="scr", bufs=2))
        small = ctx.enter_context(tc.tile_pool(name="small", bufs=1))

        # constants; block-0 phase-1 inputs are emitted first for fast start
        lhs = consts.tile([D, ROWS], dt.bfloat16)
        ones4 = consts.tile([128, D], dt.bfloat16)
        onesf = consts.tile([D, 1], dt.float32)
        xt = consts.tile([D, N], dt.bfloat16)
        auxr4 = consts.tile([128, N], dt.bfloat16)

        p1r_t = []
        p1ar_t = []
        p1al_t = []

        def load_p1(b):
            p1r = p1pool.tile([D, P1W], dt.bfloat16, tag="p1r")
            p1ar = p1pool.tile([128, P1W], dt.bfloat16, tag="p1ar")
            p1al = p1pool.tile([128, D], dt.bfloat16, tag="p1al")
            nc.sync.dma_start(p1r[:], p1r_d.ap()[b])
            for p in POS:
                nc.sync.dma_start(p1ar[p : p + 4, :], p1ar_d.ap()[b])
            nc.sync.dma_start(p1al[:], p1al_d.ap()[b])
            p1r_t.append(p1r)
            p1ar_t.append(p1ar)
            p1al_t.append(p1al)

        load_p1(0)
        load_p1(1)
        nc.sync.dma_start(lhs[:], lhs_d.ap())
        nc.sync.dma_start(ones4[:], ones4_d.ap())
        nc.sync.dma_start(onesf[:], onesf_d.ap())
        for p in POS:
            nc.sync.dma_start(auxr4[p : p + 2, :], auxr_d.ap())
        for c in range(0, N, 1024):
            nc.sync.dma_start(xt[:, c : c + 1024], xt_d.ap()[:, c : c + 1024])
        for b in range(2, BLOCKS):
            load_p1(b)

        thr = small.tile([128, BLOCKS], dt.float32)    # anchor - eps
        nthr = small.tile([128, BLOCKS], dt.float32)   # -anchor + eps (ACT bias)
        sacc = small.tile([128, 4 * BLOCKS], dt.float32)  # ACT sign sums (q*B+b)
        cacc = small.tile([128, 4 * BLOCKS], dt.float32)  # DVE gt counts (q*B+b)
        aout_t = small.tile([128, BLOCKS], dt.float32)

        ACTW = 1344  # ACT counts cols [0:ACTW), DVE counts [ACTW:2048) per chunk

        # warm the natural_log ACT table before the Sign stream (table sets
        # include the cheap functions, so one load serves Sign + Ln)
        lnwarm = small.tile([128, 1], dt.float32)
        nc.scalar.activation(lnwarm[:], onesf[:], AF.Ln)

        def phase1(b):
            lb = lhs[:, b * 128 : (b + 1) * 128]
            p1r, p1ar, p1al = p1r_t[b], p1ar_t[b], p1al_t[b]
            ps1 = psum.tile([128, CHUNK], dt.float32, tag="ps")
            for ip, p in enumerate(POS):
                c = ip * 512
                w = min(512, P1W - c)
                nc.tensor.matmul(
                    ps1[:, c : c + w],
                    lhsT=p1al[p : p + 4, :],
                    rhs=p1ar[p : p + 4, c : c + w],
                    start=True, stop=False,
                    tile_position=(p, 0),
                    skip_group_check=True,
                )
            for c in range(0, P1W, 512):
                w = min(512, P1W - c)
                nc.tensor.matmul(
                    ps1[:, c : c + w], lhsT=lb, rhs=p1r[:, c : c + w],
                    start=False, stop=True,
                    skip_group_check=True,
                )
            m8b = work.tile([128, 8], dt.float32, tag="m8")
            nc.vector.max(m8b[:], ps1[:, 0:P1W])
            nc.vector.tensor_scalar_add(thr[:, b : b + 1], m8b[:, 3:4], -EPS)
            nc.vector.tensor_scalar(
                nthr[:, b : b + 1], m8b[:, 3:4], -1.0, EPS, OP.mult, OP.add
            )
            nc.vector.tensor_copy(aout_t[:, b : b + 1], m8b[:, 3:4])

        for b in range(BLOCKS):
            phase1(b)

        for b in range(BLOCKS):
            lb = lhs[:, b * 128 : (b + 1) * 128]

            # ---- phase 2: full-row count ----
            for q in range(4):
                ps = psum.tile([128, CHUNK], dt.float32, tag="ps")
                base = q * CHUNK
                for ip, p in enumerate(POS):
                    c = ip * 512
                    nc.tensor.matmul(
                        ps[:, c : c + 512],
                        lhsT=ones4[p : p + 2, :],
                        rhs=auxr4[p : p + 2, base + c : base + c + 512],
                        start=True, stop=False,
                        tile_position=(p, 0),
                        skip_group_check=True,
                    )
                for c in range(0, CHUNK, 512):
                    nc.tensor.matmul(
                        ps[:, c : c + 512],
                        lhsT=lb, rhs=xt[:, base + c : base + c + 512],
                        start=False, stop=True,
                        skip_group_check=True,
                    )
                scr = scrp.tile([128, CHUNK], dt.bfloat16, tag="scr")
                slot = q * BLOCKS + b
                nc.scalar.activation(
                    scr[:, 0:ACTW], ps[:, 0:ACTW], AF.Sign,
                    bias=nthr[:, b : b + 1], scale=1.0,
                    accum_out=sacc[:, slot : slot + 1],
                )
                nc.vector.tensor_scalar(
                    scr[:, ACTW:CHUNK], ps[:, ACTW:CHUNK], thr[:, b : b + 1], None,
                    OP.is_gt, OP.add,
                    accum_out=cacc[:, slot : slot + 1],
                )

        # ---- m_i and digamma ----
        B = BLOCKS
        S = small.tile([128, BLOCKS], dt.float32)
        Sx = small.tile([128, BLOCKS], dt.float32)
        nc.vector.tensor_add(S[:], sacc[:, 0:B], sacc[:, B : 2 * B])
        nc.vector.tensor_add(Sx[:], sacc[:, 2 * B : 3 * B], sacc[:, 3 * B : 4 * B])
        nc.vector.tensor_add(S[:], S[:], Sx[:])
        C = small.tile([128, BLOCKS], dt.float32)
        Cx = small.tile([128, BLOCKS], dt.float32)
        nc.vector.tensor_add(C[:], cacc[:, 0:B], cacc[:, B : 2 * B])
        nc.vector.tensor_add(Cx[:], cacc[:, 2 * B : 3 * B], cacc[:, 3 * B : 4 * B])
        nc.vector.tensor_add(C[:], C[:], Cx[:])
        m = small.tile([128, BLOCKS], dt.float32)
        # m = 0.5*S + (2*ACTW - 1) + C
        nc.vector.tensor_scalar(m[:], S[:], 0.5, float(2 * ACTW - 1), OP.mult, OP.add)
        nc.vector.tensor_add(m[:], m[:], C[:])

        # digamma(m) = ln z - 1/(2z) - 1/(12 z^2) + 1/(120 z^4) - 1/(252 z^6)
        #              - 1/m - 1/(m+1) - 1/(m+2),  z = m + 3
        z = small.tile([128, BLOCKS], dt.float32)
        nc.vector.tensor_scalar_add(z[:], m[:], 3.0)
        r = small.tile([128, BLOCKS], dt.float32)
        nc.vector.reciprocal(r[:], z[:])
        r2 = small.tile([128, BLOCKS], dt.float32)
        nc.vector.tensor_mul(r2[:], r[:], r[:])
        p = small.tile([128, BLOCKS], dt.float32)
        nc.vector.tensor_scalar(p[:], r2[:], -1.0 / 252.0, 1.0 / 120.0, OP.mult, OP.add)
        u = small.tile([128, BLOCKS], dt.float32)
        nc.vector.tensor_mul(u[:], p[:], r2[:])
        nc.vector.tensor_scalar_add(u[:], u[:], -1.0 / 12.0)
        ser = small.tile([128, BLOCKS], dt.float32)
        nc.vector.tensor_mul(ser[:], u[:], r2[:])
        lnz = small.tile([128, BLOCKS], dt.float32)
        nc.scalar.activation(lnz[:], z[:], AF.Ln)
        psi = small.tile([128, BLOCKS], dt.float32)
        half_r = small.tile([128, BLOCKS], dt.float32)
        nc.vector.tensor_scalar_mul(half_r[:], r[:], 0.5)
        nc.vector.tensor_sub(psi[:], lnz[:], half_r[:])
        nc.vector.tensor_add(psi[:], psi[:], ser[:])
        w1 = small.tile([128, BLOCKS], dt.float32)
        nc.vector.tensor_scalar_add(w1[:], m[:], 1.0)
        w2 = small.tile([128, BLOCKS], dt.float32)
        nc.vector.tensor_scalar_add(w2[:], m[:], 2.0)
        rd = small.tile([128, BLOCKS], dt.float32)
        nc.vector.reciprocal(rd[:], m[:])
        nc.vector.tensor_sub(psi[:], psi[:], rd[:])
        nc.vector.reciprocal(rd[:], w1[:])
        nc.vector.tensor_sub(psi[:], psi[:], rd[:])
        nc.vector.reciprocal(rd[:], w2[:])
        nc.vector.tensor_sub(psi[:], psi[:], rd[:])

        rowsum = small.tile([128, 1], dt.float32)
        nc.vector.reduce_sum(rowsum[:], psi[:], axis=AX.X)
        pt = psum.tile([1, 1], dt.float32, tag="ps")
        nc.tensor.matmul(pt[0:1, 0:1], lhsT=rowsum[:, 0:1], rhs=onesf[:, 0:1],
                         start=True, stop=True)
        res = small.tile([1, 1], dt.float32)
        nc.vector.tensor_copy(res[:], pt[0:1, 0:1])

        nc.sync.dma_start(dsum_d.ap(), res[:])
        nc.sync.dma_start(mout_d.ap(), m[:])
        nc.sync.dma_start(aout_d.ap(), aout_t[:])

    left = _elide_redundant_waits(nc)
    assert left <= 2, f"instruction with {left} waits survived elision"
    return nc


def _elide_redundant_waits(nc):
    """Make every instruction carry <=1 semaphore wait (walrus ISA limit).

    1. Elide waits provably implied transitively by other waits (vector-clock
       pass with per-update knowledge snapshots). Only knowledge *acquired via
       waits* counts toward elision -- an engine's own completions do not (the
       CoreSim race detector, like conservative HW models, does not assume
       intra-engine issue/completion overlap is safe).
    2. Non-monotonic sems (barrier subtract) are never elided.
    3. Hoist all-but-one remaining waits onto same-engine Drain instructions
       inserted immediately before the owner.
    """
    def join(dst, src):
        for s2, v in src.items():
            if dst.get(s2, 0) < v:
                dst[s2] = v

    nonmono = set()
    for f in nc.m.functions:
        for blk in f.blocks:
            for inst in blk.instructions:
                si = inst.sync_info
                if si is None:
                    continue
                for u in si.on_update or []:
                    if u.update_mode not in ("sem-inc", "sem-add-imm") or (
                        u.update_value is not None and u.update_value < 0
                    ):
                        nonmono.add(u.ant_name)

    K_acq = {}   # proc -> knowledge acquired via waits (transitive, sound)
    K_all = {}   # proc -> K_acq + own completed updates (exported via snaps)
    snap = {}    # sem -> [(cum_value, K_all snapshot of updater)]
    cum = {}
    overloaded = []

    for f in nc.m.functions:
        for blk in f.blocks:
            for inst in blk.instructions:
                si = inst.sync_info
                if si is None:
                    continue
                waits = list(si.on_wait or [])
                updates = list(si.on_update or [])
                is_dma = inst.__class__.__name__ in ("InstDMACopy", "InstLoad", "InstSave")
                if is_dma and updates:
                    proc = "Q_" + updates[0].ant_name
                elif is_dma:
                    proc = "Q_anon_" + str(inst.name)
                else:
                    proc = "E_" + str(inst.engine)

                acq = {} if is_dma else K_acq.setdefault(proc, {})
                allk = {} if is_dma else K_all.setdefault(proc, {})

                wait_know = []
                for w in waits:
                    if w.ant_name in nonmono or w.wait_mode != "sem-ge-imm":
                        wait_know.append({})
                        continue
                    wk = {w.ant_name: w.wait_value}
                    for cv, sn in snap.get(w.ant_name, ()):
                        if cv >= w.wait_value:
                            wk = dict(sn)
                            wk[w.ant_name] = max(wk.get(w.ant_name, 0), w.wait_value)
                            break
                    wait_know.append(wk)

                kept = list(range(len(waits)))
                changed = True
                while changed:
                    changed = False
                    for idx in list(kept):
                        w = waits[idx]
                        if w.ant_name in nonmono or w.wait_mode != "sem-ge-imm":
                            continue
                        cover = dict(acq)
                        for jdx in kept:
                            if jdx != idx:
                                join(cover, wait_know[jdx])
                        if cover.get(w.ant_name, 0) >= w.wait_value:
                            kept.remove(idx)
                            changed = True

                for wk in wait_know:
                    join(acq, wk)
                    join(allk, wk)

                new_waits = [waits[i] for i in kept]
                if len(new_waits) != len(waits):
                    si.on_wait = new_waits
                    inst.sync_info = si
                if len(new_waits) > 1:
                    overloaded.append(inst)

                for u in updates:
                    s2 = u.ant_name
                    if s2 in nonmono:
                        continue
                    inc = u.update_value if u.update_value is not None else 1
                    cum[s2] = cum.get(s2, 0) + inc
                    allk[s2] = cum[s2]
                    snap.setdefault(s2, []).append((cum[s2], dict(allk)))
                if not is_dma:
                    K_acq[proc] = acq
                    K_all[proc] = allk

    if overloaded:
        import bass_rust
        import concourse.mybir as mybir

        used_ids = set()
        for f in nc.m.functions:
            for blk in f.blocks:
                for inst in blk.instructions:
                    si = inst.sync_info
                    if si is None:
                        continue
                    for w in si.on_wait or []:
                        used_ids.add(w.id)
                    for u in si.on_update or []:
                        used_ids.add(u.id)
        hsem = nc.alloc_semaphore("waithoist")
        while hsem.num in used_ids:
            hsem = nc.alloc_semaphore(f"waithoist{hsem.num}")
        over = set(id(i) for i in overloaded)
        seq = 0
        for f in nc.m.functions:
            for blk in f.blocks:
                insts = blk.instructions
                out = []
                for inst in insts:
                    if id(inst) in over:
                        si = inst.sync_info
                        waits = list(si.on_wait)
                        for w in waits[:-1]:
                            d = mybir.InstDrain(
                                name=f"WH-{seq}", ins=[], outs=[],
                                bass_is_fusable=False,
                            )
                            seq += 1
                            d.engine = inst.engine
                            d.sync_info = bass_rust.SyncInfo(
                                on_wait=[w],
                                on_update=[
                                    bass_rust.SyncUpdate(
                                        sync_type="semaphore",
                                        id=hsem.num,
                                        ant_name="waithoist",
                                        update_mode="sem-inc",
                                        update_value=1,
                                    )
                                ],
                            )
                            out.append(d)
                        inst.sync_info = bass_rust.SyncInfo(
                            on_wait=waits[-1:],
                            on_update=list(si.on_update or []),
                        )
                    out.append(inst)
                if len(out) != len(insts):
                    blk.instructions = out
    return 1


def _host_prep(X, y):
    """Class-sort + build all per-core device input tensors."""
    X = np.asarray(X, dtype=np.float32)
    y_int = np.asarray(y).astype(np.int64)

    perm = np.argsort(y_int, kind="stable")
    Xp = X[perm]
    yp = y_int[perm]
    counts = np.bincount(yp, minlength=NCLASSES)
    starts = np.zeros(NCLASSES + 1, dtype=np.int64)
    starts[1:] = np.cumsum(counts)

    XpT = np.ascontiguousarray(Xp.T)                      # [D, N] fp32
    xt_bf = XpT.astype(bf16)                              # [D, N]
    xt64 = xt_bf.astype(np.float64)
    sqv = (xt64 * xt64).sum(axis=0)                       # [N] norms of rounded pts
    sqhi = sqv.astype(bf16)
    sqlo = (sqv - sqhi.astype(np.float64)).astype(bf16)
    auxr = np.stack([-sqhi, -sqlo]).astype(bf16)          # [2, N]

    ones4 = np.zeros((128, D), dtype=bf16)
    for p in (0, 32, 64, 96):
        ones4[p : p + 2, :] = 1.0
    onesf = np.ones((D, 1), dtype=np.float32)

    in_maps = []
    for k in range(NCORES):
        rows = slice(k * ROWS, (k + 1) * ROWS)
        lhs = (2.0 * xt_bf[:, rows].astype(np.float32)).astype(bf16)  # exact 2x

        p1r = np.zeros((BLOCKS, D, P1W), dtype=bf16)
        p1ar = np.zeros((BLOCKS, 4, P1W), dtype=bf16)
        p1al = np.zeros((BLOCKS, 128, D), dtype=bf16)
        for j in range(BLOCKS):
            g0 = k * ROWS + j * 128
            cA = yp[g0]
            cB = yp[g0 + 127]
            cs = int(starts[cA])
            ce = int(starts[cB] + counts[cB])
            w = ce - cs
            assert w <= P1W
            p1r[j, :, :w] = xt_bf[:, cs:ce]
            zA = (yp[cs:ce] == cA).astype(np.float32)
            zB = (yp[cs:ce] == cB).astype(np.float32)
            p1ar[j, 0, :w] = -sqhi[cs:ce]
            p1ar[j, 0, w:] = np.float32(PADV)
            p1ar[j, 1, :w] = -sqlo[cs:ce]
            p1ar[j, 2, :w] = (1.0 - zA).astype(bf16)
            p1ar[j, 3, :w] = (1.0 - zB).astype(bf16)
            zAr = (yp[g0 : g0 + 128] == cA).astype(np.float32)
            for p in (0, 32, 64, 96):
                p1al[j, p + 0, :] = 1.0
                p1al[j, p + 1, :] = 1.0
                p1al[j, p + 2, :] = (-BIG * zAr).astype(bf16)
                p1al[j, p + 3, :] = (-BIG * (1.0 - zAr)).astype(bf16)

        in_maps.append(
            {
                "xt": xt_bf,
                "auxr": auxr,
                "lhs": lhs,
                "p1r": p1r,
                "p1ar": p1ar,
                "p1al": p1al,
                "ones4": ones4,
                "onesf": onesf,
            }
        )
    return in_maps, perm, yp, counts


def _psi_int(n):
    """digamma of a positive integer, float64."""
    n = int(n)
    g = 0.5772156649015328606
    if n < 1:
        raise ValueError(n)
    return -g + np.sum(1.0 / np.arange(1, n, dtype=np.float64))


def kernel(X, y):
    from concourse.bass_utils import run_bass_kernel_spmd

    if "nc" not in _cache:
        _cache["nc"] = _build_nc()
    nc = _cache["nc"]

    in_maps, perm, yp, counts = _host_prep(X, y)

    import os
    trace = bool(os.environ.get("BASS_TRACE"))
    results = run_bass_kernel_spmd(
        nc, in_maps, core_ids=list(range(NCORES)), trace=trace
    )
    kernel._last_results = results

    total = np.float64(0.0)
    for k in range(NCORES):
        total += np.float64(results.results[k]["dsum"][0, 0])
    avg_m = total / N

    y_int = np.asarray(y).astype(np.int64)
    Nx = np.bincount(y_int, minlength=NCLASSES)
    avg_Nx = sum((Nx[c] / N) * _psi_int(Nx[c]) for c in range(NCLASSES) if Nx[c] > 0)

    mi = _psi_int(N) - avg_Nx + _psi_int(KNN) - avg_m
    out = max(mi / np.log(2.0), 0.0)
    return np.float32(out)


kernel._last_results = None


# revision 30
# speedup vs baseline: 1.3401x; 1.3401x over previous
"""KSG mutual-information estimator (ClusterMI) on 8 Trainium2 NeuronCores.

Math (see reference):
  d2(i,j) = |x_i - x_j|^2 ; same-class 4th-smallest (k=3, self included) gives
  per-row radius; m_i = #{j : d2(i,j) <= radius_i} - 1 ;
  out = max((psi(N) - sum_c (N_c/N) psi(N_c) + psi(3) - mean_i psi(m_i)) / ln 2, 0)

Device strategy (rows sharded 1024/core, X replicated):
  Work in the s' = 2 x_i . x_j - |x_j|^2 domain: per-row ordering of s' is the
  reverse of d2's (row-constant shift), so the masked 4th-largest s' (hardware
  max8) is directly the count threshold: m_i = #{j : s'_ij > t_i - eps} - 1.
  Phase 1 per 128-row block: matmul over a ~1792-wide same-class column window
  (host-packed; cross-class pairs pushed to -BIG via a rank-2 one-hot term in
  the K=4 aux matmul), ScalarE copy PSUM->SBUF, DVE max8 -> threshold.
  Phase 2 per block: K=128 main + K=2 (-sq_hi,-sq_lo) matmuls over all 8192
  cols; counts fused into the single PSUM read: ScalarE Sign+accumulate on even
  2048-col chunks, DVE tensor_scalar(is_gt)+accumulate on odd chunks.
  Then digamma(m) by asymptotic series on-device, partition-sum via a K=128
  N=1 fp32 matmul, one scalar out per core; host combines.

bf16 matmul noise analysis: d2 noise ~0.05 abs is symmetric; count flips are
zero-mean with sigma(avg psi(m)) ~1e-4, far inside the -0.00946 pre-clamp
margin of the reference value (output is exactly 0.0 unless mi noise > 9e-3).
"""

import numpy as np
import ml_dtypes

N = 8192
D = 128
NCORES = 8
ROWS = N // NCORES          # 1024 rows per core
BLOCKS = ROWS // 128        # 8 row-blocks per core
NBLK = N // 128             # 64 global row-blocks
KNN = 3
NCLASSES = 10
P1W = 1792                  # phase-1 window width (max class pair 874+874=1748)
CHUNK = 2048                # phase-2 consumer chunk (4 PSUM banks)
MMN = 512                   # matmul free-dim per instruction
BIG = 30000.0               # cross-class penalty in s' domain
PADV = -3.0e7               # phase-1 pad-column value via -sq row
EPS = 3e-4                  # threshold shift so the anchor itself is counted

bf16 = ml_dtypes.bfloat16

_cache = {}


def _build_nc():
    from contextlib import ExitStack

    import concourse.bass as bass
    import concourse.mybir as mybir
    import concourse.tile as tile

    dt = mybir.dt
    AF = mybir.ActivationFunctionType
    OP = mybir.AluOpType
    AX = mybir.AxisListType

    nc = bass.Bass("TRN2", target_bir_lowering=False, debug=False)

    xt_d = nc.dram_tensor("xt", [D, N], dt.bfloat16, kind="ExternalInput")
    auxr_d = nc.dram_tensor("auxr", [2, N], dt.bfloat16, kind="ExternalInput")
    lhs_d = nc.dram_tensor("lhs", [D, ROWS], dt.bfloat16, kind="ExternalInput")
    p1r_d = nc.dram_tensor("p1r", [BLOCKS, D, P1W], dt.bfloat16, kind="ExternalInput")
    p1ar_d = nc.dram_tensor("p1ar", [BLOCKS, 4, P1W], dt.bfloat16, kind="ExternalInput")
    p1al_d = nc.dram_tensor("p1al", [BLOCKS, 128, D], dt.bfloat16, kind="ExternalInput")
    ones4_d = nc.dram_tensor("ones4", [128, D], dt.bfloat16, kind="ExternalInput")
    onesf_d = nc.dram_tensor("onesf", [D, 1], dt.float32, kind="ExternalInput")
    dsum_d = nc.dram_tensor("dsum", [1, 1], dt.float32, kind="ExternalOutput")
    mout_d = nc.dram_tensor("mout", [128, BLOCKS], dt.float32, kind="ExternalOutput")
    aout_d = nc.dram_tensor("aout", [128, BLOCKS], dt.float32, kind="ExternalOutput")

    POS = (0, 32, 64, 96)

    with tile.TileContext(nc) as tc, ExitStack() as ctx:
        consts = ctx.enter_context(tc.tile_pool(name="consts", bufs=1))
        p1pool = ctx.enter_context(tc.tile_pool(name="p1", bufs=4))
        psum = ctx.enter_context(tc.tile_pool(name="psum", bufs=2, space="PSUM"))
        work = ctx.enter_context(tc.tile_pool(name="work", bufs=2))
        scrp = ctx.enter_context(tc.tile_pool(name="scr", bufs=2))
        small = ctx.enter_context(tc.tile_pool(name="small", bufs=1))

        # constants; block-0 phase-1 inputs are emitted first for fast start
        lhs = consts.tile([D, ROWS], dt.bfloat16)
        ones4 = consts.tile([128, D], dt.bfloat16)
        onesf = consts.tile([D, 1], dt.float32)
        xt = consts.tile([D, N], dt.bfloat16)
        auxr4 = consts.tile([128, N], dt.bfloat16)

        p1r_t = []
        p1ar_t = []
        p1al_t = []

        def load_p1(b):
            p1r = p1pool.tile([D, P1W], dt.bfloat16, tag="p1r")
            p1ar = p1pool.tile([128, P1W], dt.bfloat16, tag="p1ar")
            p1al = p1pool.tile([128, D], dt.bfloat16, tag="p1al")
            nc.sync.dma_start(p1r[:], p1r_d.ap()[b])
            for p in POS:
                nc.sync.dma_start(p1ar[p : p + 4, :], p1ar_d.ap()[b])
            nc.sync.dma_start(p1al[:], p1al_d.ap()[b])
            p1r_t.append(p1r)
            p1ar_t.append(p1ar)
            p1al_t.append(p1al)

        load_p1(0)
        load_p1(1)
        nc.sync.dma_start(lhs[:], lhs_d.ap())
        nc.sync.dma_start(ones4[:], ones4_d.ap())
        nc.sync.dma_start(onesf[:], onesf_d.ap())
        for p in POS:
            nc.sync.dma_start(auxr4[p : p + 2, :], auxr_d.ap())
        for c in range(0, N, 1024):
            nc.sync.dma_start(xt[:, c : c + 1024], xt_d.ap()[:, c : c + 1024])
        for b in range(2, BLOCKS):
            load_p1(b)

        thr = small.tile([128, BLOCKS], dt.float32)    # anchor - eps
        nthr = small.tile([128, BLOCKS], dt.float32)   # -anchor + eps (ACT bias)
        sacc = small.tile([128, 4 * BLOCKS], dt.float32)  # ACT sign sums (q*B+b)
        cacc = small.tile([128, 4 * BLOCKS], dt.float32)  # DVE gt counts (q*B+b)
        aout_t = small.tile([128, BLOCKS], dt.float32)

        ACTW = 1344  # ACT counts cols [0:ACTW), DVE counts [ACTW:2048) per chunk

        # warm the natural_log ACT table before the Sign stream (table sets
        # include the cheap functions, so one load serves Sign + Ln)
        lnwarm = small.tile([128, 1], dt.float32)
        nc.scalar.activation(lnwarm[:], onesf[:], AF.Ln)

        def phase1(b):
            lb = lhs[:, b * 128 : (b + 1) * 128]
            p1r, p1ar, p1al = p1r_t[b], p1ar_t[b], p1al_t[b]
            ps1 = psum.tile([128, CHUNK], dt.float32, tag="ps")
            for ip, p in enumerate(POS):
                c = ip * 512
                w = min(512, P1W - c)
                nc.tensor.matmul(
                    ps1[:, c : c + w],
                    lhsT=p1al[p : p + 4, :],
                    rhs=p1ar[p : p + 4, c : c + w],
                    start=True, stop=False,
                    tile_position=(p, 0),
                    skip_group_check=True,
                )
            for c in range(0, P1W, 512):
                w = min(512, P1W - c)
                nc.tensor.matmul(
                    ps1[:, c : c + w], lhsT=lb, rhs=p1r[:, c : c + w],
                    start=False, stop=True,
                    skip_group_check=True,
                )
            m8b = work.tile([128, 8], dt.float32, tag="m8")
            nc.vector.max(m8b[:], ps1[:, 0:P1W])
            nc.vector.tensor_scalar_add(thr[:, b : b + 1], m8b[:, 3:4], -EPS)
            nc.vector.tensor_scalar(
                nthr[:, b : b + 1], m8b[:, 3:4], -1.0, EPS, OP.mult, OP.add
            )
            nc.vector.tensor_copy(aout_t[:, b : b + 1], m8b[:, 3:4])

        phase1(0)

        for b in range(BLOCKS):
            lb = lhs[:, b * 128 : (b + 1) * 128]
            if b + 1 < BLOCKS:
                phase1(b + 1)

            # ---- phase 2: full-row count ----
            for q in range(4):
                ps = psum.tile([128, CHUNK], dt.float32, tag="ps")
                base = q * CHUNK
                for ip, p in enumerate(POS):
                    c = ip * 512
                    nc.tensor.matmul(
                        ps[:, c : c + 512],
                        lhsT=ones4[p : p + 2, :],
                        rhs=auxr4[p : p + 2, base + c : base + c + 512],
                        start=True, stop=False,
                        tile_position=(p, 0),
                        skip_group_check=True,
                    )
                for c in range(0, CHUNK, 512):
                    nc.tensor.matmul(
                        ps[:, c : c + 512],
                        lhsT=lb, rhs=xt[:, base + c : base + c + 512],
                        start=False, stop=True,
                        skip_group_check=True,
                    )
                scr = scrp.tile([128, CHUNK], dt.bfloat16, tag="scr")
                slot = q * BLOCKS + b
                nc.scalar.activation(
                    scr[:, 0:ACTW], ps[:, 0:ACTW], AF.Sign,
                    bias=nthr[:, b : b + 1], scale=1.0,
                    accum_out=sacc[:, slot : slot + 1],
                )
                nc.vector.tensor_scalar(
                    scr[:, ACTW:CHUNK], ps[:, ACTW:CHUNK], thr[:, b : b + 1], None,
                    OP.is_gt, OP.add,
                    accum_out=cacc[:, slot : slot + 1],
                )

        # ---- m_i and digamma ----
        B = BLOCKS
        S = small.tile([128, BLOCKS], dt.float32)
        Sx = small.tile([128, BLOCKS], dt.float32)
        nc.vector.tensor_add(S[:], sacc[:, 0:B], sacc[:, B : 2 * B])
        nc.vector.tensor_add(Sx[:], sacc[:, 2 * B : 3 * B], sacc[:, 3 * B : 4 * B])
        nc.vector.tensor_add(S[:], S[:], Sx[:])
        C = small.tile([128, BLOCKS], dt.float32)
        Cx = small.tile([128, BLOCKS], dt.float32)
        nc.vector.tensor_add(C[:], cacc[:, 0:B], cacc[:, B : 2 * B])
        nc.vector.tensor_add(Cx[:], cacc[:, 2 * B : 3 * B], cacc[:, 3 * B : 4 * B])
        nc.vector.tensor_add(C[:], C[:], Cx[:])
        m = small.tile([128, BLOCKS], dt.float32)
        # m = 0.5*S + (2*ACTW - 1) + C
        nc.vector.tensor_scalar(m[:], S[:], 0.5, float(2 * ACTW - 1), OP.mult, OP.add)
        nc.vector.tensor_add(m[:], m[:], C[:])

        # digamma(m) = ln z - 1/(2z) - 1/(12 z^2) + 1/(120 z^4) - 1/(252 z^6)
        #              - 1/m - 1/(m+1) - 1/(m+2),  z = m + 3
        z = small.tile([128, BLOCKS], dt.float32)
        nc.vector.tensor_scalar_add(z[:], m[:], 3.0)
        r = small.tile([128, BLOCKS], dt.float32)
        nc.vector.reciprocal(r[:], z[:])
        r2 = small.tile([128, BLOCKS], dt.float32)
        nc.vector.tensor_mul(r2[:], r[:], r[:])
        p = small.tile([128, BLOCKS], dt.float32)
        nc.vector.tensor_scalar(p[:], r2[:], -1.0 / 252.0, 1.0 / 120.0, OP.mult, OP.add)
        u = small.tile([128, BLOCKS], dt.float32)
        nc.vector.tensor_mul(u[:], p[:], r2[:])
        nc.vector.tensor_scalar_add(u[:], u[:], -1.0 / 12.0)
        ser = small.tile([128, BLOCKS], dt.float32)
        nc.vector.tensor_mul(ser[:], u[:], r2[:])
        lnz = small.tile([128, BLOCKS], dt.float32)
        nc.scalar.activation(lnz[:], z[:], AF.Ln)
        psi = small.tile([128, BLOCKS], dt.float32)
        half_r = small.tile([128, BLOCKS], dt.float32)
        nc.vector.tensor_scalar_mul(half_r[:], r[:], 0.5)
        nc.vector.tensor_sub(psi[:], lnz[:], half_r[:])
        nc.vector.tensor_add(psi[:], psi[:], ser[:])
        w1 = small.tile([128, BLOCKS], dt.float32)
        nc.vector.tensor_scalar_add(w1[:], m[:], 1.0)
        w2 = small.tile([128, BLOCKS], dt.float32)
        nc.vector.tensor_scalar_add(w2[:], m[:], 2.0)
        rd = small.tile([128, BLOCKS], dt.float32)
        nc.vector.reciprocal(rd[:], m[:])
        nc.vector.tensor_sub(psi[:], psi[:], rd[:])
        nc.vector.reciprocal(rd[:], w1[:])
        nc.vector.tensor_sub(psi[:], psi[:], rd[:])
        nc.vector.reciprocal(rd[:], w2[:])
        nc.vector.tensor_sub(psi[:], psi[:], rd[:])

        rowsum = small.tile([128, 1], dt.float32)
        nc.vector.reduce_sum(rowsum[:], psi[:], axis=AX.X)
        pt = psum.tile([1, 1], dt.float32, tag="ps")
        nc.tensor.matmul(pt[0:1, 0:1], lhsT=rowsum[:, 0:1], rhs=onesf[:, 0:1],
                         start=True, stop=True)
        res = small.tile([1, 1], dt.float32)
        nc.vector.tensor_copy(res[:], pt[0:1, 0:1])

        nc.sync.dma_start(dsum_d.ap(), res[:])
        nc.sync.dma_start(mout_d.ap(), m[:])
        nc.sync.dma_start(aout_d.ap(), aout_t[:])

    left = _elide_redundant_waits(nc)
    assert left <= 2, f"instruction with {left} waits survived elision"
    return nc


def _elide_redundant_waits(nc):
    """Make every instruction carry <=1 semaphore wait (walrus ISA limit).

    1. Elide waits provably implied transitively by other waits (vector-clock
       pass with per-update knowledge snapshots). Only knowledge *acquired via
       waits* counts toward elision -- an engine's own completions do not (the
       CoreSim race detector, like conservative HW models, does not assume
       intra-engine issue/completion overlap is safe).
    2. Non-monotonic sems (barrier subtract) are never elided.
    3. Hoist all-but-one remaining waits onto same-engine Drain instructions
       inserted immediately before the owner.
    """
    def join(dst, src):
        for s2, v in src.items():
            if dst.get(s2, 0) < v:
                dst[s2] = v

    nonmono = set()
    for f in nc.m.functions:
        for blk in f.blocks:
            for inst in blk.instructions:
                si = inst.sync_info
                if si is None:
                    continue
                for u in si.on_update or []:
                    if u.update_mode not in ("sem-inc", "sem-add-imm") or (
                        u.update_value is not None and u.update_value < 0
                    ):
                        nonmono.add(u.ant_name)

    K_acq = {}   # proc -> knowledge acquired via waits (transitive, sound)
    K_all = {}   # proc -> K_acq + own completed updates (exported via snaps)
    snap = {}    # sem -> [(cum_value, K_all snapshot of updater)]
    cum = {}
    overloaded = []

    for f in nc.m.functions:
        for blk in f.blocks:
            for inst in blk.instructions:
                si = inst.sync_info
                if si is None:
                    continue
                waits = list(si.on_wait or [])
                updates = list(si.on_update or [])
                is_dma = inst.__class__.__name__ in ("InstDMACopy", "InstLoad", "InstSave")
                if is_dma and updates:
                    proc = "Q_" + updates[0].ant_name
                elif is_dma:
                    proc = "Q_anon_" + str(inst.name)
                else:
                    proc = "E_" + str(inst.engine)

                acq = {} if is_dma else K_acq.setdefault(proc, {})
                allk = {} if is_dma else K_all.setdefault(proc, {})

                wait_know = []
                for w in waits:
                    if w.ant_name in nonmono or w.wait_mode != "sem-ge-imm":
                        wait_know.append({})
                        continue
                    wk = {w.ant_name: w.wait_value}
                    for cv, sn in snap.get(w.ant_name, ()):
                        if cv >= w.wait_value:
                            wk = dict(sn)
                            wk[w.ant_name] = max(wk.get(w.ant_name, 0), w.wait_value)
                            break
                    wait_know.append(wk)

                kept = list(range(len(waits)))
                changed = True
                while changed:
                    changed = False
                    for idx in list(kept):
                        w = waits[idx]
                        if w.ant_name in nonmono or w.wait_mode != "sem-ge-imm":
                            continue
                        cover = dict(acq)
                        for jdx in kept:
                            if jdx != idx:
                                join(cover, wait_know[jdx])
                        if cover.get(w.ant_name, 0) >= w.wait_value:
                            kept.remove(idx)
                            changed = True

                for wk in wait_know:
                    join(acq, wk)
                    join(allk, wk)

                new_waits = [waits[i] for i in kept]
                if len(new_waits) != len(waits):
                    si.on_wait = new_waits
                    inst.sync_info = si
                if len(new_waits) > 1:
                    overloaded.append(inst)

                for u in updates:
                    s2 = u.ant_name
                    if s2 in nonmono:
                        continue
                    inc = u.update_value if u.update_value is not None else 1
                    cum[s2] = cum.get(s2, 0) + inc
                    allk[s2] = cum[s2]
                    snap.setdefault(s2, []).append((cum[s2], dict(allk)))
                if not is_dma:
                    K_acq[proc] = acq
                    K_all[proc] = allk

    if overloaded:
        import bass_rust
        import concourse.mybir as mybir

        used_ids = set()
        for f in nc.m.functions:
            for blk in f.blocks:
                for inst in blk.instructions:
                    si = inst.sync_info
                    if si is None:
                        continue
                    for w in si.on_wait or []:
                        used_ids.add(w.id)
                    for u in si.on_update or []:
                        used_ids.add(u.id)
        hsem = nc.alloc_semaphore("waithoist")
        while hsem.num in used_ids:
            hsem = nc.alloc_semaphore(f"waithoist{hsem.num}")
        over = set(id(i) for i in overloaded)
        seq = 0
        for f in nc.m.functions:
            for blk in f.blocks:
                insts = blk.instructions
                out = []
                for inst in insts:
                    if id(inst) in over:
                        si = inst.sync_info
                        waits = list(si.on_wait)
                        for w in waits[:-1]:
                            d = mybir.InstDrain(
                                name=f"WH-{seq}", ins=[], outs=[],
                                bass_is_fusable=False,
                            )
                            seq += 1
                            d.engine = inst.engine
                            d.sync_info = bass_rust.SyncInfo(
                                on_wait=[w],
                                on_update=[
                                    bass_rust.SyncUpdate(
                                        sync_type="semaphore",
                                        id=hsem.num,
                                        ant_name="waithoist",
                                        update_mode="sem-inc",
                                        update_value=1,
                                    )
                                ],
                            )
                            out.append(d)
                        inst.sync_info = bass_rust.SyncInfo(
                            on_wait=waits[-1:],
                            on_update=list(si.on_update or []),
                        )
                    out.append(inst)
                if len(out) != len(insts):
                    blk.instructions = out
    return 1


def _host_prep(X, y):
    """Class-sort + build all per-core device input tensors."""
    X = np.asarray(X, dtype=np.float32)
    y_int = np.asarray(y).astype(np.int64)

    perm = np.argsort(y_int, kind="stable")
    Xp = X[perm]
    yp = y_int[perm]
    counts = np.bincount(yp, minlength=NCLASSES)
    starts = np.zeros(NCLASSES + 1, dtype=np.int64)
    starts[1:] = np.cumsum(counts)

    XpT = np.ascontiguousarray(Xp.T)                      # [D, N] fp32
    xt_bf = XpT.astype(bf16)                              # [D, N]
    xt64 = xt_bf.astype(np.float64)
    sqv = (xt64 * xt64).sum(axis=0)                       # [N] norms of rounded pts
    sqhi = sqv.astype(bf16)
    sqlo = (sqv - sqhi.astype(np.float64)).astype(bf16)
    auxr = np.stack([-sqhi, -sqlo]).astype(bf16)          # [2, N]

    ones4 = np.zeros((128, D), dtype=bf16)
    for p in (0, 32, 64, 96):
        ones4[p : p + 2, :] = 1.0
    onesf = np.ones((D, 1), dtype=np.float32)

    in_maps = []
    for k in range(NCORES):
        rows = slice(k * ROWS, (k + 1) * ROWS)
        lhs = (2.0 * xt_bf[:, rows].astype(np.float32)).astype(bf16)  # exact 2x

        p1r = np.zeros((BLOCKS, D, P1W), dtype=bf16)
        p1ar = np.zeros((BLOCKS, 4, P1W), dtype=bf16)
        p1al = np.zeros((BLOCKS, 128, D), dtype=bf16)
        for j in range(BLOCKS):
            g0 = k * ROWS + j * 128
            cA = yp[g0]
            cB = yp[g0 + 127]
            cs = int(starts[cA])
            ce = int(starts[cB] + counts[cB])
            w = ce - cs
            assert w <= P1W
            p1r[j, :, :w] = xt_bf[:, cs:ce]
            zA = (yp[cs:ce] == cA).astype(np.float32)
            zB = (yp[cs:ce] == cB).astype(np.float32)
            p1ar[j, 0, :w] = -sqhi[cs:ce]
            p1ar[j, 0, w:] = np.float32(PADV)
            p1ar[j, 1, :w] = -sqlo[cs:ce]
            p1ar[j, 2, :w] = (1.0 - zA).astype(bf16)
            p1ar[j, 3, :w] = (1.0 - zB).astype(bf16)
            zAr = (yp[g0 : g0 + 128] == cA).astype(np.float32)
            for p in (0, 32, 64, 96):
                p1al[j, p + 0, :] = 1.0
                p1al[j, p + 1, :] = 1.0
                p1al[j, p + 2, :] = (-BIG * zAr).astype(bf16)
                p1al[j, p + 3, :] = (-BIG * (1.0 - zAr)).astype(bf16)

        in_maps.append(
            {
                "xt": xt_bf,
                "auxr": auxr,
                "lhs": lhs,
                "p1r": p1r,
                "p1ar": p1ar,
                "p1al": p1al,
                "ones4": ones4,
                "onesf": onesf,
            }
        )
    return in_maps, perm, yp, counts


def _psi_int(n):
    """digamma of a positive integer, float64."""
    n = int(n)
    g = 0.5772156649015328606
    if n < 1:
        raise ValueError(n)
    return -g + np.sum(1.0 / np.arange(1, n, dtype=np.float64))


def kernel(X, y):
    from concourse.bass_utils import run_bass_kernel_spmd

    if "nc" not in _cache:
        _cache["nc"] = _build_nc()
    nc = _cache["nc"]

    in_maps, perm, yp, counts = _host_prep(X, y)

    import os
    trace = bool(os.environ.get("BASS_TRACE"))
    results = run_bass_kernel_spmd(
        nc, in_maps, core_ids=list(range(NCORES)), trace=trace
    )
    kernel._last_results = results

    total = np.float64(0.0)
    for k in range(NCORES):
        total += np.float64(results.results[k]["dsum"][0, 0])
    avg_m = total / N

    y_int = np.asarray(y).astype(np.int64)
    Nx = np.bincount(y_int, minlength=NCLASSES)
    avg_Nx = sum((Nx[c] / N) * _psi_int(Nx[c]) for c in range(NCLASSES) if Nx[c] > 0)

    mi = _psi_int(N) - avg_Nx + _psi_int(KNN) - avg_m
    out = max(mi / np.log(2.0), 0.0)
    return np.float32(out)


kernel._last_results = None
